# revision 5
# baseline (speedup 1.0000x reference)
"""Trainium2 Bass kernel v2 for the MDA GNN (3x GAT views + MS-CAM fusion + pair MLP).

Same 2D distribution as v1 (core c = (a, b): a = c % 4 row quarter,
b = c // 4 output-feature half), but:

  * all big matmuls run in fp8e4m3 DoubleRow mode (2 k-tiles per pass);
    every DoubleRow operand keeps its two k-planes contiguous
    (plane stride == extent), per the ISA requirement
  * row quarters are padded to 128-multiples (CJP) so stage-1 subtiles
    and stage-2 j-tiles are always full 128/256 rows -- no tails
  * the all-gathered node-state block is fp8 with 512-byte rows:
    cols 0..451 h-half, 452..453 asrc64 as bf16 bytes, 456 ones,
    rest zero.  W is scaled x64 host-side so fp8 quantization stays
    in the normal range; the scale is undone in cheap epilogue ops
    (h cast x1/64, Exp scale=1/64)
  * wsrc/wdst enter their matmuls as fp8 hi + 16*lo column pairs,
    recovering ~bf16 accuracy for the attention logits
  * softmax weights use exp(e - 2.0) so fp8 never overflows; the shift
    cancels between numerator and row-sum
  * target rows are padded 446 -> 512 (4 uniform i-subtiles of 128);
    pt/mask tiles use an interleaved [s-block][k-plane][i'] layout so
    the matmul weight slices are contiguous
"""

import numpy as np
import ml_dtypes

import concourse.bass as bass
import concourse.mybir as mybir
import concourse.tile as tile
from concourse import bacc
from concourse.bass_utils import run_bass_kernel_spmd

F8 = mybir.dt.float8e4
BF16 = mybir.dt.bfloat16
F32 = mybir.dt.float32
AF = mybir.ActivationFunctionType
MUL = mybir.AluOpType.mult
ADD = mybir.AluOpType.add
DR = mybir.MatmulPerfMode.DoubleRow

NCORES = 8
NA = 4                # row quarters
OUT = 901
OH = 452              # half width (904 = 2*452)
WB = 512              # AG block row width (fp8 bytes)
NROWS = 1778
CI = 446              # real target rows per core
CIP = 512             # padded target rows per core (4 subtiles of 128)
NPAIRS = 4096
EPS = 1e-5
CNT = float(NROWS * OUT)
WSCALE = 64.0
WX = 456           # stage-1 moving width (452 h + asrc hi/lo + pad)

VIEWS = [
    dict(name="drug", N=2060, off=1183),
    dict(name="inc", N=2459, off=1582),
    dict(name="mrna", N=3929, off=3052),
]
for V in VIEWS:
    V["CJ"] = -(-V["N"] // NA)              # real per-core source rows
    V["NJS"] = -(-V["CJ"] // 128)           # stage-1 j subtiles
    V["CJP"] = V["NJS"] * 128               # padded per-core source rows
    V["JG"] = V["CJ"] * NA                  # gathered rows (real)
    V["NKD"] = -(-V["N"] // 256)            # double-k-tiles
    V["KP"] = V["NKD"] * 256
    V["NJT2"] = -(-V["JG"] // 256)          # stage-2 double j-tiles
    V["JGP"] = V["NJT2"] * 256              # padded gathered rows

NSUB = 4

_CACHE = {}
LAST_RESULTS = None


def _bcast(ap, parts, cols, offset=0):
    return bass.AP(tensor=ap.tensor, offset=ap.offset + offset,
                   ap=[[0, parts], [1, cols]])


def _ap3(t, rows, d1n, d1s, d2n, d2s, off=0):
    """3D AP [[partition, rows], [d1s, d1n], [d2s, d2n]] at free offset."""
    return bass.AP(tensor=t.tensor, offset=t.offset + off,
                   ap=[[t.ap[0][0], rows], [d1s, d1n], [d2s, d2n]])


def build_graph():
    nc = bacc.Bacc("TRN2", target_bir_lowering=False, debug=False,
                   enable_asserts=False, num_devices=NCORES)
    ins = {}
    for V in VIEWS:
        n = V["name"]
        ins[f"featT_{n}"] = nc.dram_tensor(f"featT_{n}", [128, V["NKD"] * V["NJS"] * 256], F8, kind="ExternalInput").ap()
        ins[f"featU_{n}"] = nc.dram_tensor(f"featU_{n}", [128, V["NKD"] * 2 * CI], F8, kind="ExternalInput").ap()
        ins[f"Wx_{n}"] = nc.dram_tensor(f"Wx_{n}", [128, V["NKD"] * 2 * WX], F8, kind="ExternalInput").ap()
        ins[f"wdhl_{n}"] = nc.dram_tensor(f"wdhl_{n}", [128, V["NKD"] * 64], F8, kind="ExternalInput").ap()
        ins[f"maskTi_{n}"] = nc.dram_tensor(f"maskTi_{n}", [V["JGP"] // 2, 2 * CIP], F8, kind="ExternalInput").ap()
        ins[f"b_{n}"] = nc.dram_tensor(f"b_{n}", [1, OH], F32, kind="ExternalInput").ap()
    ins["md"] = nc.dram_tensor("md", [CIP, OH], BF16, kind="ExternalInput").ap()
    ins["validi"] = nc.dram_tensor("validi", [CIP, 1], F32, kind="ExternalInput").ap()
    ins["camw"] = nc.dram_tensor("camw", [1, 24], F32, kind="ExternalInput").ap()
    ins["wab"] = nc.dram_tensor("wab", [2, OH], F32, kind="ExternalInput").ap()
    qr_out = nc.dram_tensor("qr", [CIP, 2], F32, kind="ExternalOutput").ap()
    rg_half = [[0, 1, 2, 3], [4, 5, 6, 7]]
    rg_all = [list(range(NCORES))]

    with tile.TileContext(nc) as tc:
        with (
            tc.tile_pool(name="persist", bufs=1) as per,
            tc.tile_pool(name="stream", bufs=2) as st,
            tc.tile_pool(name="dram", bufs=1, space="DRAM") as dr,
            tc.tile_pool(name="ps_s1", bufs=3, space="PSUM") as ps1,
            tc.tile_pool(name="ps_s2", bufs=1, space="PSUM") as ps2p,
            tc.tile_pool(name="ps_sm", bufs=1, space="PSUM") as pss,
        ):
            # ---- constants / small broadcasts ----
            ones = per.tile([128, 1], F32, tag="ones")
            nc.vector.memset(ones, 1.0)
            epst = per.tile([1, 1], F32, tag="epst")
            nc.vector.memset(epst, EPS)
            negsh = per.tile([128, 1], F32, tag="negsh")
            nc.vector.memset(negsh, -2.0)
            camb = per.tile([128, 24], F32, tag="camb")
            nc.sync.dma_start(camb, _bcast(ins["camw"], 128, 24))
            valid, invalid, mdt = {}, {}, {}
            for s in range(NSUB):
                i0 = s * 128
                valid[s] = per.tile([128, 1], F32, tag=f"valid{s}", name=f"valid{s}")
                nc.sync.dma_start(valid[s], ins["validi"][i0:i0 + 128, :])
                invalid[s] = per.tile([128, 1], F32, tag=f"invalid{s}", name=f"invalid{s}")
                nc.vector.tensor_scalar(invalid[s], valid[s],
                                        -1.0, 1.0, op0=MUL, op1=ADD)
                mdt[s] = per.tile([128, OH], BF16, tag=f"mdt{s}", name=f"mdt{s}")
                nc.sync.dma_start(mdt[s], ins["md"][i0:i0 + 128, :])

            xs = {(3, s): mdt[s] for s in range(NSUB)}
            agouts = {}
            adrs = {}
            # =================== per-view stage 1 + AG + arow ===========
            # largest view first: its (longest) AllGather starts earliest
            VORDER = (0, 1, 2)
            loaded = {}

            def load_view(vj):
                if vj in loaded:
                    return loaded[vj]
                Vj = VIEWS[vj]
                nj = Vj["name"]
                par = len(loaded) % 2
                ft = per.tile([128, Vj["NKD"] * Vj["NJS"] * 256], F8,
                              tag=f"featT{par}")
                nc.sync.dma_start(ft, ins[f"featT_{nj}"][:, :])
                wj = per.tile([128, Vj["NKD"] * 2 * WX], F8, tag="wx")
                nc.sync.dma_start(wj, ins[f"Wx_{nj}"][:, :])
                wd = per.tile([128, Vj["NKD"] * 64], F8, tag="wdhl")
                nc.sync.dma_start(wd, ins[f"wdhl_{nj}"][:, :])
                loaded[vj] = (ft, wj, wd)
                return loaded[vj]

            for oi, vi in enumerate(VORDER):
                V = VIEWS[vi]
                n, CJP, NKD, NJS = V["name"], V["CJP"], V["NKD"], V["NJS"]
                featT, wx, wdhl = load_view(vi)
                if oi + 1 < len(VORDER):
                    load_view(VORDER[oi + 1])   # prefetch next view's weights

                CJ = V["CJ"]
                agin = dr.tile([CJ, WB], F8, tag=f"agin{vi}")
                agout = dr.tile([V["JGP"], WB], F8, tag=f"agout{vi}")
                agouts[vi] = agout
                if V["JGP"] > V["JG"]:
                    zpad = V["JGP"] - V["JG"]
                    zt = st.tile([128, WB], F8, tag="zpad")
                    nc.vector.memset(zt, 0)
                    for z0 in range(0, zpad, 128):
                        zn = min(128, zpad - z0)
                        nc.sync.dma_start(agout[V["JG"] + z0: V["JG"] + z0 + zn, :],
                                          zt[:zn])

                for js in range(NJS):
                    hp = ps1.tile([128, WX], F32, tag="s1ps")
                    for dt in range(NKD):
                        nc.tensor.matmul(
                            hp,
                            _ap3(featT, 128, 2, 128, 128, 1,
                                 off=(dt * NJS + js) * 256),
                            _ap3(wx, 128, 2, WX, WX, 1, off=dt * 2 * WX),
                            start=(dt == 0), stop=(dt == NKD - 1),
                            perf_mode=DR)
                    s1out = st.tile([128, WB], F8, tag="s1out", bufs=3)
                    nc.vector.memset(s1out[:, OH:WB], 0)
                    nc.vector.memset(s1out[:, 456:457], 1.0)
                    nc.vector.tensor_scalar(s1out[:, 0:OH], hp[:, 0:OH],
                                            1.0 / WSCALE, None, op0=MUL)
                    # asrc64 = hi + lo16/16  -> bf16 bytes at cols 452..453
                    lo = st.tile([128, 1], F32, tag="s1lo")
                    nc.vector.tensor_scalar(lo, hp[:, 453:454],
                                            1.0 / 16.0, None, op0=MUL)
                    nc.vector.tensor_add(
                        s1out[:, OH:454].bitcast(BF16), hp[:, 452:453], lo)
                    pjr = min(128, CJ - js * 128)
                    nc.sync.dma_start(agin[js * 128: js * 128 + pjr, :], s1out[:pjr])

                # adst64 (hi/lo rows) for my fused i-rows: [2, CI]
                fu = per.tile([128, NKD * 2 * CI], F8, tag="fu")
                nc.sync.dma_start(fu, ins[f"featU_{n}"][:, :])
                arow = pss.tile([32, CI], F32, tag="small")
                for dt in range(NKD):
                    nc.tensor.matmul(
                        arow[:32],
                        _ap3(wdhl, 128, 2, 32, 32, 1, off=dt * 64),
                        _ap3(fu, 128, 2, CI, CI, 1, off=dt * 2 * CI),
                        start=(dt == 0), stop=(dt == NKD - 1),
                        perf_mode=DR)
                adr = dr.tile([2, CI], F32, tag=f"adr{vi}")
                arow_sb = st.tile([2, CI], F32, tag="arowsb")
                nc.vector.tensor_copy(arow_sb, arow[:2])
                nc.sync.dma_start(adr, arow_sb)
                adrs[vi] = adr

                nc.gpsimd.collective_compute(
                    "AllGather", mybir.AluOpType.bypass, replica_groups=rg_half,
                    ins=[agin.opt()], outs=[agout[0:V["JG"], :].opt()])

            # =================== per-view stage 2 =======================
            for vi in VORDER:
                V = VIEWS[vi]
                n, NJT2, JG = V["name"], V["NJT2"], V["JG"]
                agout = agouts[vi]
                ps2 = [ps2p.tile([128, WB], F32, tag=f"s2ps{s}", name=f"s2ps{s}")
                       for s in range(NSUB)]

                # adstbc = 64*adst broadcast [128, CIP]: hi_bc + lo_bc/16
                adstbc = per.tile([128, CIP], F32, tag=f"adstbc{vi}")
                nc.vector.memset(adstbc[:, CI:CIP], 0)
                lob = st.tile([128, CI], F32, tag="lob", bufs=1)
                nc.sync.dma_start(adstbc[:, 0:CI], _bcast(adrs[vi], 128, CI))
                nc.sync.dma_start(lob, _bcast(adrs[vi], 128, CI, offset=CI))
                nc.vector.scalar_tensor_tensor(
                    adstbc[:, 0:CI], lob, 1.0 / 16.0, adstbc[:, 0:CI],
                    op0=MUL, op1=ADD)

                jtch = []
                _j = 0
                while _j < NJT2:
                    take = 2 if _j + 2 <= NJT2 else 1
                    jtch.append((_j, take))
                    _j += take
                for (jt0, take) in jtch:
                    htp = st.tile([128, 4 * WB], F8, tag="ht", bufs=3)
                    nc.sync.dma_start(
                        htp[:, 0:take * 2 * WB],
                        bass.AP(tensor=agout.tensor,
                                offset=agout.offset + jt0 * 256 * WB,
                                ap=[[WB, 128], [128 * WB, 2 * take], [1, WB]]))
                    mtp = st.tile([128, 4 * CIP], F8, tag="mt", bufs=3)
                    nc.sync.dma_start(
                        mtp[:, 0:take * 2 * CIP],
                        bass.AP(tensor=ins[f"maskTi_{n}"].tensor,
                                offset=ins[f"maskTi_{n}"].offset + jt0 * 128 * 2 * CIP,
                                ap=[[2 * CIP, 128], [128 * 2 * CIP, take], [1, 2 * CIP]]))
                    for half in range(take):
                        jt = jt0 + half
                        hoff = half * 2 * WB
                        moff = half * 2 * CIP
                        # et/pt in interleaved [s-block][k-plane][i'] layout
                        et = st.tile([128, 2 * CIP], F32, tag="et", bufs=3)
                        for k in range(2):
                            asr = htp[:, hoff + k * WB + OH: hoff + k * WB + 454].bitcast(BF16)
                            nc.scalar.activation(
                                bass.AP(tensor=et.tensor, offset=et.offset + k * 128,
                                        ap=[[et.ap[0][0], 128], [256, NSUB], [1, 128]]),
                                adstbc, AF.Prelu, bias=asr, scale=1.0, alpha=0.2)
                        pt = st.tile([128, 2 * CIP], F8, tag="pt", bufs=3)
                        nc.scalar.activation(pt, et, AF.Exp,
                                             bias=negsh, scale=1.0 / WSCALE)
                        eng = nc.gpsimd if jt % 2 == 0 else nc.vector
                        eng.tensor_mul(pt, pt, mtp[:, moff:moff + 2 * CIP])
                        for s in range(NSUB):
                            nc.tensor.matmul(
                                ps2[s],
                                _ap3(pt, 128, 2, 128, 128, 1, off=s * 256),
                                _ap3(htp, 128, 2, WB, WB, 1, off=hoff),
                                start=(jt == 0), stop=(jt == NJT2 - 1),
                                perf_mode=DR)

                # epilogue: v = relu(out / rowsum + b)
                bbc = per.tile([128, OH], F32, tag="bbc", bufs=2)
                nc.sync.dma_start(bbc, _bcast(ins[f"b_{n}"], 128, OH))
                for s in range(NSUB):
                    rsum = st.tile([128, 1], F32, tag="rsum")
                    nc.vector.tensor_add(rsum, ps2[s][:, 456:457], invalid[s])
                    rs = st.tile([128, 1], F32, tag="rs")
                    nc.vector.reciprocal(rs, rsum)
                    vt = st.tile([128, OH], F32, tag="vt", bufs=2)
                    nc.vector.scalar_tensor_tensor(vt, ps2[s][:, 0:OH], rs, bbc,
                                                   op0=MUL, op1=ADD)
                    xv = per.tile([128, OH], BF16, tag=f"x{vi}{s}")
                    nc.gpsimd.tensor_relu(xv, vt)
                    xs[(vi, s)] = xv

            # =================== CAM fusion =============================
            y1 = {}
            rsums = {}
            for s in range(NSUB):
                rsums[("r1", s)] = per.tile([128, 4], F32, tag=f"rs1s{s}",
                                            name=f"rs1s{s}")
                rsums[("r2", s)] = per.tile([128, 4], F32, tag=f"rs2s{s}",
                                            name=f"rs2s{s}")
            for bi, (br, coff) in enumerate((("l", 0), ("g", 4))):
                for s in range(NSUB):
                    t = per.tile([128, OH], F32, tag=f"y1{br}{s}")
                    nc.vector.tensor_scalar_mul(t, xs[(0, s)],
                                                camb[:, coff:coff + 1])
                    for c in range(1, 3):
                        nc.vector.scalar_tensor_tensor(
                            t, xs[(c, s)], camb[:, coff + c: coff + c + 1], t,
                            op0=MUL, op1=ADD)
                    nc.vector.scalar_tensor_tensor(
                        t, xs[(3, s)], camb[:, coff + 3: coff + 4], t,
                        op0=MUL, op1=ADD,
                        accum_out=rsums[("r1", s)][:, bi:bi + 1])
                    y1[(br, s)] = t

            def stats_round(srcs, tag):
                stp = pss.tile([1, 4], F32, tag="small")
                for s in range(NSUB):
                    sc = st.tile([128, 4], F32, tag="scst", bufs=2)
                    sq = st.tile([128, OH], F32, tag="sqscr", bufs=2)
                    rst = rsums[(tag, s)]
                    for bi, br in enumerate(("l", "g")):
                        nc.scalar.activation(sq, srcs[(br, s)], AF.Square,
                                             accum_out=rst[:, 2 + bi: 3 + bi])
                    nc.vector.tensor_scalar_mul(sc, rst, valid[s])
                    nc.tensor.matmul(stp[:1], ones, sc,
                                     start=(s == 0), stop=(s == NSUB - 1))
                loc = st.tile([1, 4], F32, tag=f"loc{tag}")
                nc.vector.tensor_copy(loc, stp)
                agi = dr.tile([1, 4], F32, tag=f"sti{tag}")
                ago = dr.tile([NCORES, 4], F32, tag=f"sto{tag}", addr_space="Shared")
                nc.sync.dma_start(agi, loc)
                nc.gpsimd.collective_compute(
                    "AllGather", mybir.AluOpType.bypass, replica_groups=rg_all,
                    ins=[agi.opt()], outs=[ago.opt()])
                gsb = st.tile([NCORES, 4], F32, tag=f"gsb{tag}")
                nc.sync.dma_start(gsb, ago[:, :])
                gps = pss.tile([1, 4], F32, tag="small")
                nc.tensor.matmul(gps[:1], ones[:NCORES], gsb, start=True, stop=True)
                mrow = per.tile([1, 4], F32, tag=f"mrow{tag}")
                nc.scalar.mul(mrow, gps, 1.0 / CNT)
                m_ = mrow[0:1, 0:2]
                msq = st.tile([1, 2], F32, tag=f"msq{tag}")
                nc.vector.tensor_mul(msq, m_, m_)
                var = per.tile([1, 2], F32, tag=f"var{tag}")
                nc.vector.tensor_sub(var, mrow[0:1, 2:4], msq)
                return m_, var

            m1, var1 = stats_round(y1, "r1")
            pk1 = st.tile([1, 4], F32, tag="pk1")
            std1 = st.tile([1, 2], F32, tag="std1")
            nc.scalar.activation(std1, var1, AF.Sqrt, bias=epst[0:1, 0:1])
            nc.vector.reciprocal(pk1[:, 0:2], std1)
            nc.vector.scalar_tensor_tensor(pk1[:, 2:4], m1, -1.0, pk1[:, 0:2],
                                           op0=MUL, op1=MUL)
            d1 = dr.tile([1, 4], F32, tag="d1")
            nc.sync.dma_start(d1, pk1)
            r1bc = per.tile([128, 4], F32, tag="r1bc")
            nc.sync.dma_start(r1bc, _bcast(d1, 128, 4))
            for bi, br in enumerate(("l", "g")):
                for s in range(NSUB):
                    nc.scalar.activation(y1[(br, s)], y1[(br, s)],
                                         AF.Relu, scale=r1bc[:, bi:bi + 1],
                                         bias=r1bc[:, 2 + bi:3 + bi],
                                         accum_out=rsums[("r2", s)][:, bi:bi + 1])

            mr, vr = stats_round(y1, "r2")
            pk2 = st.tile([1, 12], F32, tag="pk2")
            den = st.tile([1, 8], F32, tag="alden")
            for bi in range(2):
                nc.vector.tensor_scalar(den[:, bi * 4:bi * 4 + 4],
                                        camb[0:1, 16 + bi * 4:20 + bi * 4],
                                        vr[0:1, bi:bi + 1], EPS,
                                        op0=MUL, op1=ADD)
            nc.scalar.activation(den, den, AF.Sqrt)
            nc.vector.reciprocal(den, den)
            for bi, coff in ((0, 8), (1, 12)):
                nc.vector.tensor_mul(pk2[:, bi * 4:bi * 4 + 4],
                                     camb[0:1, coff:coff + 4],
                                     den[0:1, bi * 4:bi * 4 + 4])
            nm = st.tile([1, 2], F32, tag="nmr2")
            nc.vector.tensor_scalar(nm, mr, -1.0, None, op0=MUL)
            bt = st.tile([1, 4], F32, tag="bt")
            nc.vector.tensor_scalar_mul(bt, pk2[:, 0:4], nm[0:1, 0:1])
            nc.vector.scalar_tensor_tensor(pk2[:, 8:12], pk2[:, 4:8],
                                           nm[0:1, 1:2], bt, op0=MUL, op1=ADD)
            d2 = dr.tile([1, 12], F32, tag="d2")
            nc.sync.dma_start(d2, pk2)
            r2bc = per.tile([128, 12], F32, tag="r2bc")
            nc.sync.dma_start(r2bc, _bcast(d2, 128, 12))

            wabc = per.tile([128, 2 * OH], F32, tag="wabc")
            nc.sync.dma_start(wabc[:, 0:OH], _bcast(ins["wab"], 128, OH, offset=0))
            nc.sync.dma_start(wabc[:, OH:2 * OH], _bcast(ins["wab"], 128, OH, offset=OH))
            for s in range(NSUB):
                acc = per.tile([128, OH], F32, tag=f"acc{s}", name=f"acc{s}")
                zp = {}
                for c in range(4):
                    zc = st.tile([128, OH], F32, tag="zc", bufs=4)
                    z2 = st.tile([128, OH], F32, tag="z2", bufs=4)
                    nc.vector.tensor_scalar(z2, y1[("l", s)],
                                            r2bc[:, c:c + 1],
                                            r2bc[:, 8 + c:9 + c],
                                            op0=MUL, op1=ADD)
                    nc.vector.scalar_tensor_tensor(
                        zc, y1[("g", s)], r2bc[:, 4 + c:5 + c], z2,
                        op0=MUL, op1=ADD)
                    nc.scalar.activation(zc, zc, AF.Sigmoid)
                    eng2 = nc.gpsimd if c % 2 == 0 else nc.vector
                    eng2.tensor_mul(z2, xs[(c, s)], zc)
                    zp[c] = z2
                nc.gpsimd.tensor_add(zp[0], zp[0], zp[1])
                nc.vector.tensor_add(zp[2], zp[2], zp[3])
                nc.vector.tensor_add(acc, zp[0], zp[2])
                qrt = st.tile([128, 2], F32, tag="qrt", bufs=2)
                zq = st.tile([128, OH], F32, tag="zq", bufs=2)
                nc.vector.tensor_mul(zq, acc, wabc[:, 0:OH])
                nc.vector.reduce_sum(qrt[:, 0:1], zq, axis=mybir.AxisListType.X)
                nc.vector.tensor_mul(zq, acc, wabc[:, OH:2 * OH])
                nc.vector.reduce_sum(qrt[:, 1:2], zq, axis=mybir.AxisListType.X)
                nc.sync.dma_start(qr_out[s * 128:(s + 1) * 128, :], qrt)
    nc.compile()
    return nc


# ======================= host side ==================================

BFH = ml_dtypes.bfloat16
F8H = ml_dtypes.float8_e4m3


def _q8(x):
    return np.clip(np.asarray(x, np.float64), -240.0, 240.0).astype(F8H)


def _rearr_k2(a, nkd, width):
    # [KP, width] -> [128, NKD*2*width], k-planes contiguous per double-tile
    kp, w = a.shape
    assert kp == nkd * 256 and w == width
    return np.ascontiguousarray(
        a.reshape(nkd, 2, 128, width).transpose(2, 0, 1, 3).reshape(128, nkd * 2 * width))


def _rearr_ft(a, nkd, njs):
    # [KP, CJP] -> [128, NKD*NJS*256]: blocks (dt, js) of [2 planes][128 j]
    kp, w = a.shape
    assert kp == nkd * 256 and w == njs * 128
    return np.ascontiguousarray(
        a.reshape(nkd, 2, 128, njs, 128).transpose(2, 0, 3, 1, 4)
         .reshape(128, nkd * njs * 256))


def _prep(inputs):
    per_core = [dict() for _ in range(NCORES)]
    frows = [np.arange(a * CI, (a + 1) * CI) for a in range(NA)]
    valids = [(fr < NROWS) for fr in frows]

    for V in VIEWS:
        n, N, off, CJ, CJP, NKD, KP, JG, NJS = (
            V["name"], V["N"], V["off"], V["CJ"], V["CJP"],
            V["NKD"], V["KP"], V["JG"], V["NJS"])
        feat = np.asarray(inputs[f"feat_{n}"], np.float32)
        adj = np.asarray(inputs[f"adj_{n}"])
        W = np.asarray(inputs[f"W_{n}"], np.float64)
        a_src = np.asarray(inputs[f"a_src_{n}"], np.float64)
        a_dst = np.asarray(inputs[f"a_dst_{n}"], np.float64)
        M = (adj != 0).astype(np.float32)
        np.fill_diagonal(M, 1.0)
        W64 = np.zeros((N, 2 * OH), np.float64)
        W64[:, :OUT] = WSCALE * W.T
        wsrc64 = np.zeros((KP,), np.float64)
        wsrc64[:N] = WSCALE * (W.T @ a_src)
        wdst64 = np.zeros((KP,), np.float64)
        wdst64[:N] = WSCALE * (W.T @ a_dst)
        bpad = np.zeros((2 * OH,), np.float32)
        bpad[:OUT] = np.asarray(inputs[f"b_{n}"], np.float32)
        feat8 = _q8(feat)

        hi_s = _q8(wsrc64)
        lo_s = _q8(16.0 * (wsrc64 - hi_s.astype(np.float64)))
        hi_d = _q8(wdst64)
        lo_d = _q8(16.0 * (wdst64 - hi_d.astype(np.float64)))
        wdhl = np.zeros((KP, 32), F8H)
        wdhl[:, 0] = hi_d
        wdhl[:, 1] = lo_d
        wdhl_r = _rearr_k2(wdhl, NKD, 32)

        featT_a, featU_a, maskTi_a = [], [], []
        for a in range(NA):
            j0, j1 = a * CJ, min((a + 1) * CJ, N)
            ft = np.zeros((KP, CJP), F8H)
            if j1 > j0:
                ft[:N, :j1 - j0] = feat8[j0:j1].T
            featT_a.append(_rearr_ft(ft, NKD, NJS))
            fr, va = frows[a], valids[a]
            vrow = np.where(fr < OUT, fr, off + fr - OUT)[va]
            fu = np.zeros((KP, CI), F8H)
            fu[:N, :vrow.size] = feat8[vrow].T
            featU_a.append(_rearr_k2(fu, NKD, CI))
            # gathered row r == global j; interleave [JGP//2, 2*CIP]
            JGP = V["JGP"]
            mg = np.zeros((JGP, CIP), np.float32)
            mg[:N, :vrow.size] = M[:, vrow]
            mi = mg.reshape(JGP // 256, 2, 128, NSUB, 128) \
                   .transpose(0, 2, 3, 1, 4).reshape(JGP // 2, 2 * CIP)
            maskTi_a.append(mi.astype(F8H))
        for c in range(NCORES):
            a, b = c % NA, c // NA
            Wx = np.zeros((KP, WX), F8H)
            Wx[:N, 0:OH] = _q8(W64[:, b * OH:(b + 1) * OH])
            Wx[:, OH] = hi_s
            Wx[:, 453] = lo_s
            per_core[c][f"Wx_{n}"] = _rearr_k2(Wx, NKD, WX)
            per_core[c][f"wdhl_{n}"] = wdhl_r
            per_core[c][f"featT_{n}"] = featT_a[a]
            per_core[c][f"featU_{n}"] = featU_a[a]
            per_core[c][f"maskTi_{n}"] = maskTi_a[a]
            per_core[c][f"b_{n}"] = bpad[b * OH:(b + 1) * OH].reshape(1, OH)

    # collapsed pair-MLP vector + constant
    mW1 = np.asarray(inputs["mW1"], np.float64)
    mW2 = np.asarray(inputs["mW2"], np.float64)
    mW3 = np.asarray(inputs["mW3"], np.float64)
    mW4 = np.asarray(inputs["mW4"], np.float64)
    w432 = mW4 @ mW3 @ mW2
    wfull = (w432 @ mW1)[0]
    cconst = (np.asarray(inputs["mb1"], np.float64) @ w432[0]
              + np.asarray(inputs["mb2"], np.float64) @ (mW4 @ mW3)[0]
              + np.asarray(inputs["mb3"], np.float64) @ mW4[0]
              + np.asarray(inputs["mb4"], np.float64)[0])
    wap = np.zeros((2 * OH,), np.float64)
    wap[:OUT] = wfull[:OUT] / 4.0
    wbp = np.zeros((2 * OH,), np.float64)
    wbp[:OUT] = wfull[OUT:] / 4.0

    lw2r = np.asarray(inputs["lw2"], np.float32).ravel()
    gw2r = np.asarray(inputs["gw2"], np.float32).ravel()
    camw = np.concatenate([
        np.asarray(inputs["lw1"], np.float32).ravel(),
        np.asarray(inputs["gw1"], np.float32).ravel(),
        lw2r, gw2r, lw2r * lw2r, gw2r * gw2r]).reshape(1, 24)

    md = np.asarray(inputs["mirna_disease"], np.float32)
    mdp = np.zeros((NA * CIP, 2 * OH), np.float32)
    for a in range(NA):
        fr, va = frows[a], valids[a]
        mdp[a * CIP: a * CIP + int(va.sum()), :OUT] = md[fr[va]]
    for c in range(NCORES):
        a, b = c % NA, c // NA
        per_core[c]["md"] = mdp[a * CIP:(a + 1) * CIP, b * OH:(b + 1) * OH].astype(BFH)
        vp = np.zeros((CIP,), np.float32)
        vp[:CI] = valids[a].astype(np.float32)
        per_core[c]["validi"] = vp.reshape(CIP, 1)
        per_core[c]["camw"] = camw
        per_core[c]["wab"] = np.stack(
            [wap[b * OH:(b + 1) * OH], wbp[b * OH:(b + 1) * OH]]).astype(np.float32)
    return per_core, float(cconst)


def kernel(**inputs):
    global LAST_RESULTS
    if "nc" not in _CACHE:
        _CACHE["nc"] = build_graph()
    nc = _CACHE["nc"]
    in_maps, cconst = _prep(inputs)
    res = run_bass_kernel_spmd(nc, in_maps, core_ids=list(range(NCORES)))
    LAST_RESULTS = res
    qr_halves = [np.concatenate([np.asarray(res.results[b * NA + a]["qr"])[:CI]
                                 for a in range(NA)]) for b in range(2)]
    qr = qr_halves[0] + qr_halves[1]
    q, r = qr[:NROWS, 0], qr[:NROWS, 1]
    ts = np.asarray(inputs["test_sample"])
    out = (q[ts[:, 0]] + r[ts[:, 1]] + cconst).astype(np.float32)
    return out.reshape(NPAIRS, 1)
